# revision 11
# baseline (speedup 1.0000x reference)
"""Trainium2 Bass kernel for CombinedLSTMWithStatic2Hop.

Model: per-node LSTM over T timesteps + static encoder -> fusion -> 2x SAGEConv
(mean aggregation) -> linear head.

Sharding: B*N = 8000 nodes split into 1000 contiguous nodes per core (8 cores).
Each graph (2000 nodes) spans a core pair; SAGE aggregation uses pairwise
AllGather of node features between the two halves.

LSTM layout: hidden/gate dim on partitions, nodes on the free dim, two node
chunks of 500 ping-ponging over 2x4 PSUM banks.  All transcendentals are Tanh
(sigmoid(x) = 0.5*(tanh(x/2)+1) folded algebraically):
  state s := 2c, hh := 2h.
  PSUM_g = 0.5*W_hh[g] @ hh + W_ih[g] @ x_t + b[g]        (= preact for i,f,o)
  PSUM_g(gate g) scaled 2x via host-side weight scaling    (= 2*preact for g)
  T = tanh(0.5 * PSUM)            one ACT op per chunk  [i|f|o|g]
  P2 = (Ti + 1) * Tg              = 2*i*g                 (DVE fused stt)
  Q2 = (Tf + 1) * s               = 4*f*c                 (DVE fused stt)
  s' = 0.5*Q2 + P2                = 2*c'                  (DVE fused stt)
  Tc = tanh(0.5 * s')             = tanh(c')              (ACT)
  hh' = (To + 1) * Tc             = 2*h'                  (DVE fused stt)
Biases enter through a ones-row on the x-side matmul (K=9).
The trailing 2x of hh is folded into W_fuse's z-half on the host.
"""

import os as _os

import numpy as np

import concourse.bass as bass
import concourse.tile as tile
from concourse import bacc, mybir
from concourse.bass_utils import run_bass_kernel_spmd
from concourse.masks import make_identity

F32 = mybir.dt.float32

B, T, N, E = 4, 96, 2000, 16000
F_DYN, F_STA, H = 8, 16, 128
N_CORES = 8
NPC = B * N // N_CORES      # 1000 nodes per core
CH = NPC // 2               # 500 node chunk
GSL = 512                   # psum gate slot stride (one bank)

# module-level knobs (test.py may override)
TRACE = False
TRACE_KW = {}

_PROG_CACHE = {}


def _build_program(t_steps=T):
    nc = bacc.Bacc("TRN2", target_bir_lowering=False, debug=False,
                   num_devices=N_CORES)

    assert t_steps % 3 == 0
    tb = t_steps // 3  # x blocks of 3 timesteps (partition groups 0/32/64)

    # ---- DRAM I/O ----
    x_dram = nc.dram_tensor("x_dyn", [tb * 27, NPC], F32, kind="ExternalInput")
    w_rec = nc.dram_tensor("w_rec", [H, 4 * H], F32, kind="ExternalInput")
    w_x = nc.dram_tensor("w_x", [128, 4 * H], F32, kind="ExternalInput")
    w_sta = nc.dram_tensor("w_sta", [F_STA + 1, H], F32, kind="ExternalInput")
    sta_t = nc.dram_tensor("sta_t", [F_STA + 1, NPC], F32, kind="ExternalInput")
    w_fz = nc.dram_tensor("w_fz", [H, H], F32, kind="ExternalInput")
    w_fs = nc.dram_tensor("w_fs", [H, H], F32, kind="ExternalInput")
    b_fu = nc.dram_tensor("b_fu", [H, 1], F32, kind="ExternalInput")
    w_r1 = nc.dram_tensor("w_r1", [H, H], F32, kind="ExternalInput")
    w_l1 = nc.dram_tensor("w_l1", [H, H], F32, kind="ExternalInput")
    b_l1 = nc.dram_tensor("b_l1", [H, 1], F32, kind="ExternalInput")
    w_r2 = nc.dram_tensor("w_r2", [H, H], F32, kind="ExternalInput")
    w_l2 = nc.dram_tensor("w_l2", [H, H], F32, kind="ExternalInput")
    b_l2 = nc.dram_tensor("b_l2", [H, 1], F32, kind="ExternalInput")
    w_ou = nc.dram_tensor("w_ou", [H, 1], F32, kind="ExternalInput")
    b_ou = nc.dram_tensor("b_ou", [1, 1], F32, kind="ExternalInput")
    a_mat = nc.dram_tensor("a_mat", [N, NPC], F32, kind="ExternalInput")
    out_d = nc.dram_tensor("out", [1, NPC], F32, kind="ExternalOutput")

    AT = mybir.AluOpType
    AF = mybir.ActivationFunctionType
    n_kchunks = (N + 127) // 128  # 16 src chunks for aggregation

    with tile.TileContext(nc) as tc:
        with (
            tc.tile_pool(name="const", bufs=1) as cp,
            tc.tile_pool(name="xp", bufs=3) as xp,
            tc.tile_pool(name="wk", bufs=2) as wk,
        ):
            # ---- constants into SBUF ----
            def cload(dram, shape, tag):
                tl = cp.tile(shape, F32, tag=tag)
                nc.sync.dma_start(out=tl[:, :], in_=dram[:, :])
                return tl

            w_rec_t = cload(w_rec, [H, 4 * H], "w_rec")
            w_x_t = cload(w_x, [128, 4 * H], "w_x")
            w_sta_tt = cload(w_sta, [F_STA + 1, H], "w_sta")
            sta_tt = cload(sta_t, [F_STA + 1, NPC], "sta_t")
            w_fz_t = cload(w_fz, [H, H], "w_fz")
            w_fs_t = cload(w_fs, [H, H], "w_fs")
            b_fu_t = cload(b_fu, [H, 1], "b_fu")
            w_r1_t = cload(w_r1, [H, H], "w_r1")
            w_l1_t = cload(w_l1, [H, H], "w_l1")
            b_l1_t = cload(b_l1, [H, 1], "b_l1")
            w_r2_t = cload(w_r2, [H, H], "w_r2")
            w_l2_t = cload(w_l2, [H, H], "w_l2")
            b_l2_t = cload(b_l2, [H, 1], "b_l2")
            w_ou_t = cload(w_ou, [H, 1], "w_ou")
            b_ou_t = cload(b_ou, [1, 1], "b_ou")

            a_tiles = []
            for k in range(n_kchunks):
                mk = min(128, N - 128 * k)
                tl = cp.tile([128, NPC], F32, tag=f"a{k}")
                nc.sync.dma_start(out=tl[0:mk, :], in_=a_mat[128 * k:128 * k + mk, :])
                a_tiles.append((tl, mk))

            ident = cp.tile([128, 128], F32, tag="ident")
            make_identity(nc, ident[:, :])

            # ---- LSTM ----
            # persistent psum gate tiles: [i|f|o|g] gate slots at 512 strides
            pl_cm = tc.tile_pool(name="psl", bufs=2, space="PSUM")
            pl = pl_cm.__enter__()
            ps_ch = [pl.tile([128, 4 * GSL], F32, tag="gates", name="gates0"),
                     pl.tile([128, 4 * GSL], F32, tag="gates", name="gates1")]
            # zero the pad columns the gate ACT op reads ([500:512] of each slot)
            for ps in ps_ch:
                nc.vector.memset(ps[:, :], 0.0)

            def load_xblock(b):
                tl = xp.tile([128, NPC], F32, tag="xb")
                for g in range(3):
                    nc.sync.dma_start(
                        out=tl[32 * g:32 * g + 9, :],
                        in_=x_dram[27 * b + 9 * g:27 * b + 9 * g + 9, :])
                return tl

            xtiles = {0: load_xblock(0)}
            if tb > 1:
                xtiles[1] = load_xblock(1)

            s_prev = None
            h_prev = None
            for t in range(t_steps):
                blk, grp = divmod(t, 3)
                if grp == 0 and blk + 2 < tb:
                    xtiles[blk + 2] = load_xblock(blk + 2)
                xt = xtiles[blk]

                s_new = wk.tile([128, NPC], F32, tag="s")
                h_new = wk.tile([128, NPC], F32, tag="h")
                tc_t = wk.tile([128, NPC], F32, tag="tc", bufs=1)

                for c in range(2):
                    ps = ps_ch[c]
                    sl = slice(CH * c, CH * c + CH)
                    tt = wk.tile([128, 3 * GSL + CH], F32, tag=f"T{c}")

                    for gi in range(4):
                        osl = slice(GSL * gi, GSL * gi + CH)
                        if t > 0:
                            nc.tensor.matmul(
                                out=ps[:, osl],
                                lhsT=w_rec_t[:, H * gi:H * gi + H],
                                rhs=h_prev[:, sl],
                                start=True, stop=False)
                        nc.tensor.matmul(
                            out=ps[:, osl],
                            lhsT=w_x_t[32 * grp:32 * grp + 9, H * gi:H * gi + H],
                            rhs=xt[32 * grp:32 * grp + 9, sl],
                            start=(t == 0), stop=True)

                    # T = tanh(0.5 * psum) over [i|f|o|g] incl pad cols
                    nc.scalar.activation(
                        out=tt[:, 0:3 * GSL + CH], in_=ps[:, 0:3 * GSL + CH],
                        func=AF.Tanh, scale=0.5)

                    ti = tt[:, 0:CH]
                    tf = tt[:, GSL:GSL + CH]
                    to = tt[:, 2 * GSL:2 * GSL + CH]
                    tg = tt[:, 3 * GSL:3 * GSL + CH]

                    if t > 0:
                        q2 = wk.tile([128, CH], F32, tag=f"q{c}", bufs=1)
                        p2 = wk.tile([128, CH], F32, tag=f"p{c}", bufs=1)
                        # q2 = (Tf + 1) * s_prev ; p2 = (Ti + 1) * Tg
                        nc.vector.scalar_tensor_tensor(
                            out=q2[:, :], in0=tf, scalar=1.0, in1=s_prev[:, sl],
                            op0=AT.add, op1=AT.mult)
                        nc.vector.scalar_tensor_tensor(
                            out=p2[:, :], in0=ti, scalar=1.0, in1=tg,
                            op0=AT.add, op1=AT.mult)
                        # s' = 0.5*q2 + p2
                        nc.vector.scalar_tensor_tensor(
                            out=s_new[:, sl], in0=q2[:, :], scalar=0.5, in1=p2[:, :],
                            op0=AT.mult, op1=AT.add)
                    else:
                        # s0 = (Ti + 1) * Tg
                        nc.vector.scalar_tensor_tensor(
                            out=s_new[:, sl], in0=ti, scalar=1.0, in1=tg,
                            op0=AT.add, op1=AT.mult)

                    # Tc = tanh(0.5 * s') = tanh(c')
                    nc.scalar.activation(
                        out=tc_t[:, sl], in_=s_new[:, sl], func=AF.Tanh, scale=0.5)
                    # hh = (To + 1) * Tc
                    nc.vector.scalar_tensor_tensor(
                        out=h_new[:, sl], in0=to, scalar=1.0, in1=tc_t[:, sl],
                        op0=AT.add, op1=AT.mult)

                s_prev, h_prev = s_new, h_new

            hh = h_prev  # [128, NPC] = 2 * h_final
            if _os.environ.get("K_SKIP_GNN"):
                pred0 = wk.tile([1, NPC], F32, tag="pred0", bufs=1)
                nc.vector.tensor_copy(out=pred0[0:1, :], in_=hh[0:1, :])
                nc.sync.dma_start(out=out_d[0:1, :], in_=pred0[0:1, :])
                pl_cm.__exit__(None, None, None)
                return nc
            pl_cm.__exit__(None, None, None)
            pp_cm = tc.tile_pool(name="psg", bufs=2, space="PSUM")
            pp = pp_cm.__enter__()

            # ---- static encoder + fusion ----
            def mm_pair(psum, pairs):
                # pairs: (lhsT_ap, rhs_tile, rhs_partitions); accumulates in psum
                # output halves: [0:CH] at slot 0, [GSL:GSL+CH] at slot 1
                for c in range(2):
                    osl = slice(GSL * c, GSL * c + CH)
                    for j, (lt, rtile, pr) in enumerate(pairs):
                        nc.tensor.matmul(
                            out=psum[:, osl], lhsT=lt,
                            rhs=rtile[0:pr, CH * c:CH * c + CH],
                            start=(j == 0), stop=(j == len(pairs) - 1))

            def psum_to_sbuf_act(psum, dst, func, bias=0.0):
                for c in range(2):
                    nc.scalar.activation(
                        out=dst[:, CH * c:CH * c + CH],
                        in_=psum[:, GSL * c:GSL * c + CH],
                        func=func, bias=bias, scale=1.0)

            stl = wk.tile([128, NPC], F32, tag="stl", bufs=1)
            pss = pp.tile([128, 2 * GSL], F32, tag="gp")
            mm_pair(pss, [(w_sta_tt[0:17, :], sta_tt, 17)])
            psum_to_sbuf_act(pss, stl, AF.Relu)

            node_t = wk.tile([128, NPC], F32, tag="node", bufs=1)
            psf = pp.tile([128, 2 * GSL], F32, tag="gp")
            mm_pair(psf, [(w_fz_t[:, :], hh, 128), (w_fs_t[:, :], stl, 128)])
            psum_to_sbuf_act(psf, node_t, AF.Relu, bias=b_fu_t[:, 0:1])

            # ---- SAGE layers ----
            with tc.tile_pool(name="dram", bufs=1, space="DRAM") as dp:
                def sage(x_t_tile, w_r, w_l, b_l, relu, lname):
                    # transpose x^T [128, NPC] -> node-major [NPC, 128] in SBUF
                    x_nm = wk.tile([128, 8 * 128], F32, tag="xnm", bufs=1)
                    for k in range((NPC + 127) // 128):
                        nk = min(128, NPC - 128 * k)
                        trp = pp.tile([128, 128], F32, tag="tr")
                        nc.tensor.transpose(
                            out=trp[0:nk, :],
                            in_=x_t_tile[:, 128 * k:128 * k + nk],
                            identity=ident[:, :])
                        nc.vector.tensor_copy(
                            out=x_nm[0:nk, 128 * k:128 * k + 128],
                            in_=trp[0:nk, :])

                    cc_in = dp.tile([NPC, H], F32, tag=f"ci{lname}")
                    cc_out = dp.tile([2 * NPC, H], F32, tag=f"co{lname}")
                    for k in range((NPC + 127) // 128):
                        nk = min(128, NPC - 128 * k)
                        nc.sync.dma_start(
                            out=cc_in[128 * k:128 * k + nk, :],
                            in_=x_nm[0:nk, 128 * k:128 * k + 128])
                    if _os.environ.get("K_SKIP_CC"):
                        nc.sync.dma_start(out=cc_out[0:NPC, :], in_=cc_in[:, :])
                        nc.sync.dma_start(out=cc_out[NPC:2 * NPC, :],
                                          in_=cc_in[:, :])
                    else:
                        nc.gpsimd.collective_compute(
                            "AllGather", AT.bypass,
                            replica_groups=[[0, 1], [2, 3], [4, 5], [6, 7]],
                            ins=[cc_in.opt()], outs=[cc_out.opt()])

                    xfull = wk.tile([128, n_kchunks * 128], F32, tag="xfull", bufs=1)
                    for k in range(n_kchunks):
                        mk = min(128, N - 128 * k)
                        nc.sync.dma_start(
                            out=xfull[0:mk, 128 * k:128 * k + 128],
                            in_=cc_out[128 * k:128 * k + mk, :])

                    # mean^T = sum_k xfull_k^T @ A_k
                    psm = pp.tile([128, 2 * GSL], F32, tag="gp")
                    for c in range(2):
                        osl = slice(GSL * c, GSL * c + CH)
                        for k in range(n_kchunks):
                            atile, mk = a_tiles[k]
                            nc.tensor.matmul(
                                out=psm[:, osl],
                                lhsT=xfull[0:mk, 128 * k:128 * k + 128],
                                rhs=atile[0:mk, CH * c:CH * c + CH],
                                start=(k == 0), stop=(k == n_kchunks - 1))
                    mean_t = wk.tile([128, NPC], F32, tag="mean", bufs=1)
                    for c in range(2):
                        nc.vector.tensor_copy(
                            out=mean_t[:, CH * c:CH * c + CH],
                            in_=psm[:, GSL * c:GSL * c + CH])

                    psh = pp.tile([128, 2 * GSL], F32, tag="gp")
                    mm_pair(psh, [(w_r[:, :], x_t_tile, 128),
                                  (w_l[:, :], mean_t, 128)])
                    out_t = wk.tile([128, NPC], F32, tag=f"o{lname}", bufs=1)
                    psum_to_sbuf_act(psh, out_t,
                                     AF.Relu if relu else AF.Identity,
                                     bias=b_l[:, 0:1])
                    return out_t

                h1_t = sage(node_t, w_r1_t, w_l1_t, b_l1_t, True, "1")
                h2_t = sage(h1_t, w_r2_t, w_l2_t, b_l2_t, False, "2")

                # ---- head ----
                pso = pp.tile([1, 2 * GSL], F32, tag="ho", bufs=1)
                for c in range(2):
                    nc.tensor.matmul(
                        out=pso[0:1, GSL * c:GSL * c + CH],
                        lhsT=w_ou_t[:, 0:1],
                        rhs=h2_t[:, CH * c:CH * c + CH],
                        start=True, stop=True)
                pred = wk.tile([1, NPC], F32, tag="pred", bufs=1)
                for c in range(2):
                    nc.vector.tensor_scalar(
                        out=pred[0:1, CH * c:CH * c + CH],
                        in0=pso[0:1, GSL * c:GSL * c + CH],
                        scalar1=b_ou_t[0:1, 0:1], scalar2=None, op0=AT.add)
                nc.sync.dma_start(out=out_d[0:1, :], in_=pred[0:1, :])
            pp_cm.__exit__(None, None, None)

    return nc


def _prep_inputs(inputs, t_steps=T):
    """Host-side preprocessing: per-core input maps."""
    dyn = np.asarray(inputs["dynamic_features"], np.float32)
    sta = np.asarray(inputs["static_features"], np.float32)
    ei = np.asarray(inputs["edge_index"])
    W_ih = np.asarray(inputs["W_ih"], np.float32)
    W_hh = np.asarray(inputs["W_hh"], np.float32)
    b = (np.asarray(inputs["b_ih"], np.float32)
         + np.asarray(inputs["b_hh"], np.float32))
    W_sta = np.asarray(inputs["W_sta"], np.float32)
    b_sta = np.asarray(inputs["b_sta"], np.float32)
    W_fuse = np.asarray(inputs["W_fuse"], np.float32)
    b_fuse = np.asarray(inputs["b_fuse"], np.float32)
    s1_Wl = np.asarray(inputs["sage1_Wl"], np.float32)
    s1_bl = np.asarray(inputs["sage1_bl"], np.float32)
    s1_Wr = np.asarray(inputs["sage1_Wr"], np.float32)
    s2_Wl = np.asarray(inputs["sage2_Wl"], np.float32)
    s2_bl = np.asarray(inputs["sage2_bl"], np.float32)
    s2_Wr = np.asarray(inputs["sage2_Wr"], np.float32)
    W_out = np.asarray(inputs["W_out"], np.float32)
    b_out = np.asarray(inputs["b_out"], np.float32)

    tb = t_steps // 3

    # gate order in psum: [i, f, o, g]; torch order in weights: i,f,g,o
    gsl = [slice(0, H), slice(H, 2 * H), slice(3 * H, 4 * H), slice(2 * H, 3 * H)]
    # w_rec: lhsT [h_in, 4H], scaled 0.5 (hh=2h), g-gate additionally 2x
    w_rec = np.concatenate(
        [0.5 * W_hh[gsl[0]].T, 0.5 * W_hh[gsl[1]].T,
         0.5 * W_hh[gsl[2]].T, 1.0 * W_hh[gsl[3]].T], axis=1).astype(np.float32)
    # w_x: [9, 4H] = [W_ih^T; bias row], g-gate 2x; replicated at 4x32 partitions
    wx9 = np.zeros((9, 4 * H), np.float32)
    for j, s in enumerate(gsl):
        sc = 2.0 if j == 3 else 1.0
        wx9[0:8, H * j:H * j + H] = sc * W_ih[s].T
        wx9[8, H * j:H * j + H] = sc * b[s]
    w_x = np.zeros((128, 4 * H), np.float32)
    for g in range(3):
        w_x[32 * g:32 * g + 9] = wx9

    w_sta_t = np.zeros((F_STA + 1, H), np.float32)
    w_sta_t[0:F_STA] = W_sta.T
    w_sta_t[F_STA] = b_sta

    w_fz = np.ascontiguousarray((0.5 * W_fuse[:, :H]).T)
    w_fs = np.ascontiguousarray(W_fuse[:, H:].T)

    # normalized adjacency (same graph for every batch element)
    src, dst = ei[0].astype(np.int64), ei[1].astype(np.int64)
    cnt = np.bincount(dst, minlength=N).astype(np.float32)
    A = np.zeros((N, N), np.float32)
    np.add.at(A, (src, dst), 1.0)
    A /= np.maximum(cnt, 1.0)[None, :]

    x_bn = dyn.transpose(0, 2, 1, 3).reshape(B * N, dyn.shape[1], F_DYN)
    sta_bn = sta.reshape(B * N, F_STA)

    shared = dict(
        w_rec=w_rec, w_x=w_x, w_sta=w_sta_t,
        w_fz=w_fz, w_fs=w_fs, b_fu=b_fuse.reshape(H, 1),
        w_r1=np.ascontiguousarray(s1_Wr.T), w_l1=np.ascontiguousarray(s1_Wl.T),
        b_l1=s1_bl.reshape(H, 1),
        w_r2=np.ascontiguousarray(s2_Wr.T), w_l2=np.ascontiguousarray(s2_Wl.T),
        b_l2=s2_bl.reshape(H, 1),
        w_ou=np.ascontiguousarray(W_out.T), b_ou=b_out.reshape(1, 1),
    )

    in_maps = []
    for core in range(N_CORES):
        rows = slice(NPC * core, NPC * core + NPC)
        xc = x_bn[rows, 0:t_steps, :]                       # [NPC, T, 8]
        xt = xc.transpose(1, 2, 0)                          # [T, 8, NPC]
        arr = np.ones((tb, 3, 9, NPC), np.float32)
        arr[:, :, 0:8, :] = xt.reshape(tb, 3, 8, NPC)
        x_in = np.ascontiguousarray(arr.reshape(tb * 27, NPC))

        sta_in = np.ones((F_STA + 1, NPC), np.float32)
        sta_in[0:F_STA] = sta_bn[rows].T

        half = core % 2
        a_in = np.ascontiguousarray(A[:, NPC * half:NPC * half + NPC])

        m = dict(shared)
        m.update(x_dyn=x_in, sta_t=sta_in, a_mat=a_in)
        in_maps.append(m)
    return in_maps


def kernel(**inputs):
    t_steps = int(np.asarray(inputs["dynamic_features"]).shape[1])
    if t_steps not in _PROG_CACHE:
        nc_new = _build_program(t_steps)
        if not nc_new.is_finalized():
            nc_new.finalize()
        _PROG_CACHE[t_steps] = nc_new
    nc = _PROG_CACHE[t_steps]
    in_maps = _prep_inputs(inputs, t_steps)
    br = run_bass_kernel_spmd(nc, in_maps, list(range(N_CORES)),
                              trace=TRACE, **TRACE_KW)
    kernel.last_result = br
    out = np.concatenate(
        [np.asarray(br.results[c]["out"]).reshape(NPC) for c in range(N_CORES)])
    return out.reshape(B, N).astype(np.float32)
